# revision 4
# baseline (speedup 1.0000x reference)
"""MoE expert-routing kernel for Trainium2 (8 NeuronCores).

out[b] = x[b] @ weight[index[b]] + bias[index[b]]

Expert-parallel (4 experts/core), host-side token routing and host-side
bias add (the einsum runs on device; bias is O(B*D) host work like the
pack/unpack transposes). Per-core slot order is [smallest, largest,
2nd-largest, 3rd-largest] expert by count: a small slot0 starts the PE
earliest, a small-ish slot3 shortens the output tail; slot capacity
caps[s] = max over cores at that position (rounded to 8) keeps the
program SPMD.

Device schedule (per core):
  - inputs: Sync ring [blk0, blk2], Scalar ring [blk1, blk3]; all four
    in flight immediately; compute consumes in arrival order.
  - warm-up matmuls on TensorE during the DMA wait release the HAM
    clock throttle (1.2 -> 2.4 GHz) before the real matmuls.
  - compute slot s: 2 o-halves x 2 K-half accumulating matmuls, N=caps[s].
  - PSUM drains (plain fp32->fp16 copies, VectorE tensor_scalar_add 0.0)
    -- no scalar-engine activations, so no ACT_TABLE_LOAD stall.
  - outputs: slots 0/1 as one DMA each, slots 2/3 split per o-half,
    rings alternating, so the final DMA is small and early.
"""

import numpy as np

B, E, DIN, DOUT = 4096, 32, 256, 256
NCORES = 8
EPC = E // NCORES

TRACE = False
LAST_RESULT = None
NWARM = 7

_PROGRAM_CACHE = {}


def _build_program(caps):
    import concourse.bass as bass
    import concourse.mybir as mybir
    import concourse.tile as tile
    from concourse import bacc

    f32 = mybir.dt.float32
    f16 = mybir.dt.float16

    xoff = 4 * 128
    CK = 512                  # PSUM f32 bank limit
    OUTW = sum(2 * c for c in caps)
    ooff = np.concatenate([[0], np.cumsum([2 * c for c in caps])])

    nc = bacc.Bacc("TRN2", target_bir_lowering=False, debug=False,
                   enable_asserts=False)

    blk_d = [nc.dram_tensor(f"blk{s}", [128, xoff + 2 * caps[s]], f16,
                            kind="ExternalInput") for s in range(EPC)]
    out_d = nc.dram_tensor("out", [128, OUTW], f16, kind="ExternalOutput")

    warm_src = nc.alloc_sbuf_tensor("warm_src", [128, CK], f16).ap()

    with tile.TileContext(nc) as tc:
        with (
            tc.tile_pool(name="bin", bufs=4) as bpool,
            tc.tile_pool(name="oout", bufs=4) as opool,
            tc.tile_pool(name="psum", bufs=6, space=bass.MemorySpace.PSUM)
                as ppool,
            tc.tile_pool(name="warm", bufs=1, space=bass.MemorySpace.PSUM)
                as wpool,
        ):
            # input DMAs: both HWDGE rings loaded immediately
            blks = [bpool.tile([128, xoff + 2 * caps[s]], f16,
                               name=f"blk{s}", tag=f"blk{s}", bufs=1)
                    for s in range(EPC)]
            nc.sync.dma_start(blks[0][:], blk_d[0].ap())
            nc.scalar.dma_start(blks[1][:], blk_d[1].ap())
            nc.sync.dma_start(blks[2][:], blk_d[2].ap())
            nc.scalar.dma_start(blks[3][:], blk_d[3].ap())

            # PE warm-up from uninitialized raw SBUF (never read back):
            # no memset, no cross-engine sem, starts at body entry.
            wp = wpool.tile([128, CK], f32)
            for i in range(NWARM):
                nc.tensor.matmul(wp[:], warm_src[:, :128], warm_src[:],
                                 start=True, stop=True)

            for s in range(EPC):
                cap = caps[s]
                blk = blks[s]
                ot = opool.tile([128, 2 * cap], f16, name=f"ot{s}",
                                tag=f"ot{s}", bufs=1)
                for oh in range(2):
                    if s == EPC - 1 and oh == 1 and 64 < cap <= CK:
                        chunks = [(0, cap - 64), (cap - 64, 64)]
                    else:
                        chunks = [(ck, min(CK, cap - ck))
                                  for ck in range(0, cap, CK)]
                    for ck, cw in chunks:
                        ps = ppool.tile([128, CK], f32, name=f"ps{s}_{oh}",
                                        tag="ps")
                        for k in range(2):
                            nc.tensor.matmul(
                                ps[:, :cw],
                                blk[:, (k * 2 + oh) * 128:
                                    (k * 2 + oh + 1) * 128],
                                blk[:, xoff + k * cap + ck:
                                    xoff + k * cap + ck + cw],
                                start=(k == 0), stop=(k == 1),
                            )
                        nc.vector.tensor_scalar_add(
                            ot[:, oh * cap + ck:oh * cap + ck + cw],
                            ps[:, :cw], 0.0)
                o0, o1 = int(ooff[s]), int(ooff[s + 1])
                if s == EPC - 1 and 64 < cap <= CK:
                    cut = 2 * cap - 64
                    nc.scalar.dma_start(out_d.ap()[:, o0:o0 + cut],
                                        ot[:, :cut])
                    nc.sync.dma_start(out_d.ap()[:, o0 + cut:o1],
                                      ot[:, cut:])
                else:
                    eng = [nc.sync, nc.scalar, nc.sync, nc.scalar][s]
                    eng.dma_start(out_d.ap()[:, o0:o1], ot[:])

    nc.compile()
    return nc


def _route(index):
    order = np.argsort(index, kind="stable")
    counts = np.bincount(index, minlength=E)
    offs = np.zeros(E + 1, np.int64)
    offs[1:] = np.cumsum(counts)
    # per-core experts sorted desc by count, then placed into slots as
    # [smallest, largest, 2nd, 3rd]
    PERM = [2, 0, 1, 3]                        # desc-rank for each slot
    slot_experts = np.empty((NCORES, EPC), np.int64)
    for c in range(NCORES):
        ce = np.arange(c * EPC, (c + 1) * EPC)
        desc = ce[np.argsort(-counts[ce], kind="stable")]
        slot_experts[c] = desc[PERM]
    sorted_counts = counts[slot_experts]          # [NCORES, EPC]
    caps = tuple(int(-(-int(m) // 8) * 8) if m else 8
                 for m in sorted_counts.max(axis=0))
    return order, counts, offs, slot_experts, caps


def _pack_core(x16, w16, order, offs, slot_experts, caps, c):
    xoff = 4 * 128
    blks = {}
    for s in range(EPC):
        cap = caps[s]
        e = slot_experts[c, s]
        blk = np.zeros((128, xoff + 2 * cap), np.float16)
        toks = order[offs[e]:offs[e + 1]]
        xT = x16[toks].T
        for k in range(2):
            for oh in range(2):
                blk[:, (k * 2 + oh) * 128:(k * 2 + oh + 1) * 128] = \
                    w16[e, k * 128:(k + 1) * 128, oh * 128:(oh + 1) * 128]
            blk[:, xoff + k * cap:xoff + k * cap + xT.shape[1]] = \
                xT[k * 128:(k + 1) * 128]
        blks[f"blk{s}"] = np.ascontiguousarray(blk)
    return blks


def kernel(x, index, weight, bias):
    from concourse.bass_utils import run_bass_kernel_spmd

    global LAST_RESULT

    x = np.asarray(x, np.float32)
    index = np.asarray(index, np.int32)
    weight = np.asarray(weight, np.float32)
    bias = np.asarray(bias, np.float32)

    order, counts, offs, slot_experts, caps = _route(index)

    if caps not in _PROGRAM_CACHE:
        _PROGRAM_CACHE[caps] = _build_program(caps)
    nc = _PROGRAM_CACHE[caps]

    x16 = x.astype(np.float16)
    w16 = weight.astype(np.float16)
    in_maps = [_pack_core(x16, w16, order, offs, slot_experts, caps, c)
               for c in range(NCORES)]

    kwargs = {}
    if TRACE:
        kwargs = dict(trace=True, trace_cores=list(range(NCORES)))
    res = run_bass_kernel_spmd(nc, in_maps, core_ids=list(range(NCORES)),
                               **kwargs)
    LAST_RESULT = res

    ooff = np.concatenate([[0], np.cumsum([2 * c for c in caps])])
    out = np.empty((B, DOUT), np.float32)
    for c in range(NCORES):
        oc = res.results[c]["out"]  # [128, OUTW] fp16
        for s in range(EPC):
            cap = caps[s]
            e = slot_experts[c, s]
            toks = order[offs[e]:offs[e + 1]]
            oe = oc[:, ooff[s]:ooff[s + 1]].reshape(128, 2, cap)
            # [p, oh, t] -> [t, oh*128+p]
            oe = oe.transpose(2, 1, 0).reshape(cap, DOUT)
            out[toks] = oe[:len(toks)].astype(np.float32) + bias[e]
    return out


# revision 5
# speedup vs baseline: 1.0038x; 1.0038x over previous
"""MoE expert-routing kernel for Trainium2 (8 NeuronCores).

out[b] = x[b] @ weight[index[b]] + bias[index[b]]

Expert-parallel (4 experts/core), host-side token routing and host-side
bias add (the einsum runs on device; bias is O(B*D) host work like the
pack/unpack transposes). Per-core slot order is [smallest, largest,
2nd-largest, 3rd-largest] expert by count: a small slot0 starts the PE
earliest, a small-ish slot3 shortens the output tail; slot capacity
caps[s] = max over cores at that position (rounded to 8) keeps the
program SPMD.

Device schedule (per core):
  - inputs: Sync ring [blk0, blk2], Scalar ring [blk1, blk3]; all four
    in flight immediately; compute consumes in arrival order.
  - warm-up matmuls on TensorE during the DMA wait release the HAM
    clock throttle (1.2 -> 2.4 GHz) before the real matmuls.
  - compute slot s: 2 o-halves x 2 K-half accumulating matmuls, N=caps[s].
  - PSUM drains (plain fp32->fp16 copies, VectorE tensor_scalar_add 0.0)
    -- no scalar-engine activations, so no ACT_TABLE_LOAD stall.
  - outputs: slots 0/1 as one DMA each, slots 2/3 split per o-half,
    rings alternating, so the final DMA is small and early.
"""

import numpy as np

B, E, DIN, DOUT = 4096, 32, 256, 256
NCORES = 8
EPC = E // NCORES

TRACE = False
LAST_RESULT = None
NWARM = 8

_PROGRAM_CACHE = {}


def _build_program(caps):
    import concourse.bass as bass
    import concourse.mybir as mybir
    import concourse.tile as tile
    from concourse import bacc

    f32 = mybir.dt.float32
    f16 = mybir.dt.float16

    xoff = 4 * 128
    CK = 512                  # PSUM f32 bank limit
    OUTW = sum(2 * c for c in caps)
    ooff = np.concatenate([[0], np.cumsum([2 * c for c in caps])])

    nc = bacc.Bacc("TRN2", target_bir_lowering=False, debug=False,
                   enable_asserts=False)

    blk_d = [nc.dram_tensor(f"blk{s}", [128, xoff + 2 * caps[s]], f16,
                            kind="ExternalInput") for s in range(EPC)]
    out_d = nc.dram_tensor("out", [128, OUTW], f16, kind="ExternalOutput")

    warm_src = nc.alloc_sbuf_tensor("warm_src", [128, CK], f16).ap()

    with tile.TileContext(nc) as tc:
        with (
            tc.tile_pool(name="bin", bufs=4) as bpool,
            tc.tile_pool(name="oout", bufs=4) as opool,
            tc.tile_pool(name="psum", bufs=6, space=bass.MemorySpace.PSUM)
                as ppool,
            tc.tile_pool(name="warm", bufs=1, space=bass.MemorySpace.PSUM)
                as wpool,
        ):
            # input DMAs: both HWDGE rings loaded immediately
            blks = [bpool.tile([128, xoff + 2 * caps[s]], f16,
                               name=f"blk{s}", tag=f"blk{s}", bufs=1)
                    for s in range(EPC)]
            nc.sync.dma_start(blks[0][:], blk_d[0].ap())
            nc.scalar.dma_start(blks[1][:], blk_d[1].ap())
            nc.sync.dma_start(blks[2][:], blk_d[2].ap())
            nc.scalar.dma_start(blks[3][:], blk_d[3].ap())

            # PE warm-up from uninitialized raw SBUF (never read back):
            # no memset, no cross-engine sem, starts at body entry.
            wp = wpool.tile([128, CK], f32)
            for i in range(NWARM):
                nc.tensor.matmul(wp[:], warm_src[:, :128], warm_src[:],
                                 start=True, stop=True)

            for s in range(EPC):
                cap = caps[s]
                blk = blks[s]
                ot = opool.tile([128, 2 * cap], f16, name=f"ot{s}",
                                tag=f"ot{s}", bufs=1)
                for oh in range(2):
                    if s == EPC - 1 and oh == 1 and 64 < cap <= CK:
                        chunks = [(0, cap - 64), (cap - 64, 64)]
                    else:
                        chunks = [(ck, min(CK, cap - ck))
                                  for ck in range(0, cap, CK)]
                    for ck, cw in chunks:
                        ps = ppool.tile([128, CK], f32, name=f"ps{s}_{oh}",
                                        tag="ps")
                        for k in range(2):
                            nc.tensor.matmul(
                                ps[:, :cw],
                                blk[:, (k * 2 + oh) * 128:
                                    (k * 2 + oh + 1) * 128],
                                blk[:, xoff + k * cap + ck:
                                    xoff + k * cap + ck + cw],
                                start=(k == 0), stop=(k == 1),
                            )
                        nc.vector.tensor_scalar_add(
                            ot[:, oh * cap + ck:oh * cap + ck + cw],
                            ps[:, :cw], 0.0)
                o0, o1 = int(ooff[s]), int(ooff[s + 1])
                if s == EPC - 1 and 64 < cap <= CK:
                    cut = 2 * cap - 64
                    nc.scalar.dma_start(out_d.ap()[:, o0:o0 + cut],
                                        ot[:, :cut])
                    nc.sync.dma_start(out_d.ap()[:, o0 + cut:o1],
                                      ot[:, cut:])
                else:
                    eng = [nc.sync, nc.scalar, nc.sync, nc.scalar][s]
                    eng.dma_start(out_d.ap()[:, o0:o1], ot[:])

    nc.compile()
    return nc


def _route(index):
    order = np.argsort(index, kind="stable")
    counts = np.bincount(index, minlength=E)
    offs = np.zeros(E + 1, np.int64)
    offs[1:] = np.cumsum(counts)
    # per-core experts sorted desc by count, then placed into slots as
    # [smallest, largest, 2nd, 3rd]
    PERM = [2, 0, 1, 3]                        # desc-rank for each slot
    slot_experts = np.empty((NCORES, EPC), np.int64)
    for c in range(NCORES):
        ce = np.arange(c * EPC, (c + 1) * EPC)
        desc = ce[np.argsort(-counts[ce], kind="stable")]
        slot_experts[c] = desc[PERM]
    sorted_counts = counts[slot_experts]          # [NCORES, EPC]
    caps = tuple(int(-(-int(m) // 8) * 8) if m else 8
                 for m in sorted_counts.max(axis=0))
    return order, counts, offs, slot_experts, caps


def _pack_core(x16, w16, order, offs, slot_experts, caps, c):
    xoff = 4 * 128
    blks = {}
    for s in range(EPC):
        cap = caps[s]
        e = slot_experts[c, s]
        blk = np.zeros((128, xoff + 2 * cap), np.float16)
        toks = order[offs[e]:offs[e + 1]]
        xT = x16[toks].T
        for k in range(2):
            for oh in range(2):
                blk[:, (k * 2 + oh) * 128:(k * 2 + oh + 1) * 128] = \
                    w16[e, k * 128:(k + 1) * 128, oh * 128:(oh + 1) * 128]
            blk[:, xoff + k * cap:xoff + k * cap + xT.shape[1]] = \
                xT[k * 128:(k + 1) * 128]
        blks[f"blk{s}"] = np.ascontiguousarray(blk)
    return blks


def kernel(x, index, weight, bias):
    from concourse.bass_utils import run_bass_kernel_spmd

    global LAST_RESULT

    x = np.asarray(x, np.float32)
    index = np.asarray(index, np.int32)
    weight = np.asarray(weight, np.float32)
    bias = np.asarray(bias, np.float32)

    order, counts, offs, slot_experts, caps = _route(index)

    if caps not in _PROGRAM_CACHE:
        _PROGRAM_CACHE[caps] = _build_program(caps)
    nc = _PROGRAM_CACHE[caps]

    x16 = x.astype(np.float16)
    w16 = weight.astype(np.float16)
    in_maps = [_pack_core(x16, w16, order, offs, slot_experts, caps, c)
               for c in range(NCORES)]

    kwargs = {}
    if TRACE:
        kwargs = dict(trace=True, trace_cores=list(range(NCORES)))
    res = run_bass_kernel_spmd(nc, in_maps, core_ids=list(range(NCORES)),
                               **kwargs)
    LAST_RESULT = res

    ooff = np.concatenate([[0], np.cumsum([2 * c for c in caps])])
    out = np.empty((B, DOUT), np.float32)
    for c in range(NCORES):
        oc = res.results[c]["out"]  # [128, OUTW] fp16
        for s in range(EPC):
            cap = caps[s]
            e = slot_experts[c, s]
            toks = order[offs[e]:offs[e + 1]]
            oe = oc[:, ooff[s]:ooff[s + 1]].reshape(128, 2, cap)
            # [p, oh, t] -> [t, oh*128+p]
            oe = oe.transpose(2, 1, 0).reshape(cap, DOUT)
            out[toks] = oe[:len(toks)].astype(np.float32) + bias[e]
    return out
